# revision 17
# baseline (speedup 1.0000x reference)
"""Trainium2 Bass kernel for an MQA attention block (8 q-heads, shared K/V).

Sharding: 8 cores; core c -> batch b=c//4, query rows s0=(c%4)*512 .. +512,
all 8 heads.  K/V (full sequence, per batch) computed redundantly per core.

v2 design notes:
- All LN affines folded on host: rms1 into Wq/Wk/Wv, q/k gammas (and DQ^-0.5)
  into the rope cos/sin tables, v gamma/beta through attention into Wo/bo,
  rms2 into Wo/bo.
- LN stats via ScalarE: the PSUM->SBUF drain copy doubles as the row-sum
  (activation Identity + accum_out), a Square activation gives sum-of-squares.
- Attention bias is preloaded into PSUM by the PE itself (matmul with an
  identity stationary and the bias as moving operand, start=True), so the
  QK matmuls accumulate on top -- no DVE bias add.
- Rope'd q/k, softmax probs, v rows, y and Wo all run in bf16 (PE full rate
  at any moving width); projections and logits accumulate in fp32.
- Softmax denominator from a ones-column appended to v (no max subtraction
  needed: logits softcapped to +-5).
"""

import os
import sys

for _p in ("/opt/trn_rl_repo",):
    if _p not in sys.path and os.path.isdir(_p):
        sys.path.insert(0, _p)

import numpy as np
from contextlib import ExitStack

import concourse.bass as bass
import concourse.mybir as mybir
import concourse.tile as tile
from concourse import bacc
from concourse import bass_utils

F32 = mybir.dt.float32
F32R = mybir.dt.float32r
BF16 = mybir.dt.bfloat16

B, S, D = 2, 2048, 1536
H, DQ, DK, DV = 8, 128, 128, 192
P = 128
SQ = S // 4          # 512 query rows per core
DC = D // P          # 12 contraction chunks
JC = S // P          # 16 key chunks
SC = SQ // P         # 4 query-row chunks
NCORES = 8
EPS_RMS = 1e-6
EPS_LN = 1e-5
SOFTCAP = 5.0
ROPE_BASE = 8192.0
HALF = DQ // 2
VW = 256             # vrow inner stride (bf16); cols 0:192 v, 192 ones
JH = S // 2          # columns per half in kv projection


def _r(ap):
    return ap.bitcast(F32R)


def build_program(has_rbq=False, has_rbk=False):
    nc = bacc.Bacc(
        "TRN2", target_bir_lowering=False, debug=False, num_devices=NCORES
    )

    def din(name, shape, dt=F32):
        return nc.dram_tensor(name, list(shape), dt, kind="ExternalInput").ap()

    # per-core inputs (host pre-arranged to device layouts, contiguous)
    xT = din("xT", (D, S))
    xTq = din("xTq", (P, DC, SQ))
    biasT = din("biasT", (P, JC, SQ), BF16)
    c1q_t = din("c1q", (P, SC, HALF), BF16)
    s2nq_t = din("s2nq", (P, SC, HALF), BF16)
    s1q_t = din("s1q", (P, SC, HALF), BF16)
    c2q_t = din("c2q", (P, SC, HALF), BF16)
    # shared (replicated) inputs
    c1k_t = din("c1k", (P, JC, HALF), BF16)
    s2nk_t = din("s2nk", (P, JC, HALF), BF16)
    s1k_t = din("s1k", (P, JC, HALF), BF16)
    c2k_t = din("c2k", (P, JC, HALF), BF16)
    wq = din("wq", (P, DC, H * DQ))
    wk = din("wk", (P, DC, DK))
    wva_t = din("wva", (P, DC, P))
    wvb_t = din("wvb", (P, DC, DV - P))
    wo = din("wo", (P, DC, D), BF16)
    packf_t = din("packf", (P, 140))  # identf|bk|bvA|bvB|bqh packed
    bor_t = din("bor", (P, D))    # row-replicated output bias (all folds)
    identb_t = din("identb", (P, P), BF16)
    if has_rbq:
        rbq_t = din("rbq", (P, SC, DQ), BF16)
    if has_rbk:
        rbk_t = din("rbk", (P, JC, DK), BF16)
    out = nc.dram_tensor("out", [SQ, D], F32, kind="ExternalOutput").ap()

    TT = mybir.AluOpType
    AF = mybir.ActivationFunctionType
    AX = mybir.AxisListType

    with tile.TileContext(nc) as tc, ExitStack() as ctx:
        const = ctx.enter_context(tc.tile_pool(name="const", bufs=1))
        persist = ctx.enter_context(tc.tile_pool(name="persist", bufs=1))
        scrp = ctx.enter_context(
            tc.tile_pool(name="scrp", bufs=2, space="PSUM")
        )

        # ---- constants (DMAs emitted after the kv weight DMAs)
        packf = const.tile([P, 140], F32)
        bk_sb = packf[:, 128:129]
        bvA = packf[:, 129:130]
        bvB = packf[0 : DV - P, 130:131]
        bqh_sb = packf[:, 131:139]
        identb = const.tile([P, P], BF16)
        eps_sb = const.tile([P, 1], F32)
        nc.vector.memset(eps_sb[:], EPS_LN)

        def load_tab(t, n, nm):
            tt = const.tile([P, n, HALF], BF16, tag=nm, name=nm)
            nc.sync.dma_start(tt[:], t)
            return tt

        if has_rbk:
            rbk = const.tile([P, JC, DK], BF16)
            nc.sync.dma_start(rbk[:], rbk_t)
        if has_rbq:
            rbq = const.tile([P, SC, DQ], BF16)
            nc.sync.dma_start(rbq[:], rbq_t)


        # persistent activations
        kT_sb = persist.tile([P, S], BF16)           # rope'd k, [dk, s]
        vrow_sb = persist.tile([P, JC, VW], BF16)    # v rows + ones col
        nc.vector.memset(vrow_sb[:, :, DV : DV + 1], 1.0)
        qT = [
            persist.tile([P, SQ], BF16, tag=f"q{h}", name=f"qT{h}")
            for h in range(H)
        ]
        yp = [
            persist.tile([P, SC, 2 * DV], BF16, tag=f"yp{p}", name=f"yp{p}")
            for p in range(4)
        ]

        # =========================================================
        # KV phase
        # =========================================================
        def ln_stats(pool, rows, G, W, grp, nm):
            """bn_stats/aggr over [P, G, W] rows -> (agg [P,G,2], rst [P,G,1])
            grp chunks per bn_stats call (grp*W <= 512)."""
            st6 = pool.tile([P, G, 6], F32, tag=nm + "st6", name=nm + "st6")
            agg = pool.tile([P, G, 2], F32, tag=nm + "agg", name=nm + "agg")
            for t in range(G):
                nc.vector.bn_stats(st6[:, t, :], rows[:, t, :])
                nc.vector.bn_aggr(agg[:, t, :], st6[:, t, :])
            rst = pool.tile([P, G, 1], F32, tag=nm + "rst", name=nm + "rst")
            nc.scalar.activation(
                rst[:], agg[:, :, 1:2], AF.Sqrt, bias=eps_sb[:, 0:1]
            )
            nc.vector.reciprocal(rst[:], rst[:])
            return agg, rst

        with (
            tc.tile_pool(name="kvw", bufs=1) as kvw,
            tc.tile_pool(name="kvx", bufs=3) as kvx,
            tc.tile_pool(name="kvd", bufs=2) as kvd,
            tc.tile_pool(name="kvt", bufs=2) as kvt,
            tc.tile_pool(name="kvps", bufs=1, space="PSUM") as kvps,
        ):
            wk_sb = kvw.tile([P, DC, DK], F32)
            nc.sync.dma_start(_r(wk_sb[:]), _r(wk))
            wva = kvw.tile([P, DC, P], F32)
            wvb = kvw.tile([P, DC, DV - P], F32)
            nc.sync.dma_start(_r(wva[:]), _r(wva_t))
            nc.sync.dma_start(_r(wvb[:]), _r(wvb_t))
            nc.sync.dma_start(packf[:], packf_t)
            nc.sync.dma_start(identb[:], identb_t)

            krs = []
            for jh in range(2):
                j0 = jh * JH
                kps = kvps.tile([P, JH], F32, tag="kps")
                vaps = kvps.tile([P, JH], F32, tag="vaps")
                vbps = kvps.tile([DV - P, JH], F32, tag="vbps")
                for dc in range(DC):
                    xt = kvx.tile([P, JH], F32, tag="xt")
                    nc.sync.dma_start(
                        _r(xt[:]), _r(xT[dc * P : (dc + 1) * P, j0 : j0 + JH])
                    )
                    for n in range(JH // 512):
                        cs = slice(n * 512, (n + 1) * 512)
                        nc.tensor.matmul(
                            kps[:, cs], _r(wk_sb[:, dc, :]), _r(xt[:, cs]),
                            start=(dc == 0), stop=(dc == DC - 1),
                        )
                        nc.tensor.matmul(
                            vaps[:, cs], _r(wva[:, dc, :]), _r(xt[:, cs]),
                            start=(dc == 0), stop=(dc == DC - 1),
                        )
                        nc.tensor.matmul(
                            vbps[:, cs], _r(wvb[:, dc, :]), _r(xt[:, cs]),
                            start=(dc == 0), stop=(dc == DC - 1),
                        )

                if jh == 0:
                    c1k = load_tab(c1k_t, JC, "c1k")
                    s2nk = load_tab(s2nk_t, JC, "s2nk")
                    s1k = load_tab(s1k_t, JC, "s1k")
                    c2k = load_tab(c2k_t, JC, "c2k")

                # drains with folded rms1 bias
                k_sb = kvd.tile([P, JH], BF16, tag="k_sb")
                nc.scalar.activation(
                    k_sb[:], kps[:], AF.Identity, bias=bk_sb
                )
                va_sb = kvd.tile([P, JH], BF16, tag="va_sb")
                nc.scalar.activation(
                    va_sb[:], vaps[:], AF.Identity, bias=bvA
                )
                vb_sb = kvd.tile([DV - P, JH], BF16, tag="vb_sb")
                nc.scalar.activation(
                    vb_sb[:], vbps[:], AF.Identity, bias=bvB
                )

                # ---- k chunks: transpose -> rows, LN stats, rope
                krow = kvt.tile([P, 8, DK], BF16, tag="krow")
                for t in range(8):
                    tsl = slice(t * P, (t + 1) * P)
                    scr = scrp.tile([P, P], BF16, tag="scr", name=f"sk{jh}{t}")
                    nc.tensor.transpose(scr[:], k_sb[:, tsl], identb[:])
                    nc.scalar.activation(krow[:, t, :], scr[:], AF.Identity)
                kagg, krst = ln_stats(kvt, krow, 8, DK, 4, "k")
                xn = kvt.tile([P, 8, DK], BF16, tag="xn")
                for t in range(8):
                    nc.vector.tensor_scalar(
                        xn[:, t, :], krow[:, t, :],
                        kagg[:, t, 0:1], krst[:, t, 0:1],
                        TT.subtract, TT.mult,
                    )
                # batched rope over the 8 chunks
                jcs = slice(jh * 8, jh * 8 + 8)
                z1 = kvt.tile([P, 8, HALF], BF16, tag="z1")
                z2 = kvt.tile([P, 8, HALF], BF16, tag="z2")
                kr = kvt.tile([P, 8, DK], BF16, tag="kr")
                nc.vector.tensor_tensor(
                    z1[:], xn[:, :, :HALF], c1k[:, jcs, :], TT.mult
                )
                nc.vector.tensor_tensor(
                    z2[:], xn[:, :, HALF:], s2nk[:, jcs, :], TT.mult
                )
                nc.vector.tensor_tensor(kr[:, :, :HALF], z1[:], z2[:], TT.add)
                nc.vector.tensor_tensor(
                    z1[:], xn[:, :, :HALF], s1k[:, jcs, :], TT.mult
                )
                nc.vector.tensor_tensor(
                    z2[:], xn[:, :, HALF:], c2k[:, jcs, :], TT.mult
                )
                nc.vector.tensor_tensor(kr[:, :, HALF:], z1[:], z2[:], TT.add)
                if has_rbk:
                    nc.vector.tensor_tensor(
                        kr[:], kr[:], rbk[:, jcs, :], TT.add
                    )
                krs.append(kr)

                # ---- v chunks
                vrowf = kvt.tile([P, 8, DV], BF16, tag="vrowf")
                for t in range(8):
                    tsl = slice(t * P, (t + 1) * P)
                    scv = scrp.tile([P, P + 64], BF16, tag="scr",
                                    name=f"sv{jh}{t}")
                    nc.tensor.transpose(scv[:, :P], va_sb[:, tsl], identb[:])
                    nc.tensor.transpose(
                        scv[:, P:], vb_sb[:, tsl],
                        identb[: DV - P, : DV - P],
                    )
                    nc.scalar.activation(
                        vrowf[:, t, :], scv[:, :DV], AF.Identity
                    )
                vagg, vrst = ln_stats(kvt, vrowf, 8, DV, 2, "v")
                for t in range(8):
                    jc = jh * 8 + t
                    nc.vector.tensor_scalar(
                        vrow_sb[:, jc, :DV], vrowf[:, t, :],
                        vagg[:, t, 0:1], vrst[:, t, 0:1],
                        TT.subtract, TT.mult,
                    )

            # deferred k back-transposes (keeps the PE stream dense)
            for jh in range(2):
                for t in range(8):
                    jc = jh * 8 + t
                    scb = scrp.tile([P, P], BF16, tag="scr", name=f"sb{jh}{t}")
                    nc.tensor.transpose(scb[:], krs[jh][:, t, :], identb[:])
                    nc.vector.tensor_copy(
                        kT_sb[:, jc * P : (jc + 1) * P], scb[:]
                    )

        # attention constants loaded during Q phase
        wop = ctx.enter_context(tc.tile_pool(name="wop", bufs=1))
        biasT_sb = wop.tile([P, JC, SQ], BF16)
        nc.sync.dma_start(biasT_sb[:], biasT)

        # =========================================================
        # Q phase: software-pipelined stages so PE stays dense
        # =========================================================
        with (
            tc.tile_pool(name="qx", bufs=1) as qx,
            tc.tile_pool(name="qw", bufs=2) as qw,
            tc.tile_pool(name="qt", bufs=2) as qt,
            tc.tile_pool(name="qps", bufs=2, space="PSUM") as qps,
            tc.tile_pool(name="qscr", bufs=2, space="PSUM") as qscr,
        ):
            xtq_sb = qx.tile([P, DC, SQ], F32)
            nc.sync.dma_start(_r(xtq_sb[:]), _r(xTq))
            c1q = load_tab(c1q_t, SC, "c1q")
            s2nq = load_tab(s2nq_t, SC, "s2nq")
            s1q = load_tab(s1q_t, SC, "s1q")
            c2q = load_tab(c2q_t, SC, "c2q")

            def q_stageA(h):
                wqh = qw.tile([P, DC, DQ], F32, tag="wqh", name=f"wqh{h}")
                nc.sync.dma_start(
                    _r(wqh[:]), _r(wq[:, :, h * DQ : (h + 1) * DQ])
                )
                q_ps = qps.tile([P, SQ], F32, tag="q_ps", name=f"qps{h}")
                for dc in range(DC):
                    nc.tensor.matmul(
                        q_ps[:], _r(wqh[:, dc, :]), _r(xtq_sb[:, dc, :]),
                        start=(dc == 0), stop=(dc == DC - 1),
                    )
                q_sb = qt.tile([P, SQ], BF16, tag="q_sb", name=f"qsb{h}")
                nc.scalar.activation(
                    q_sb[:], q_ps[:], AF.Identity, bias=bqh_sb[:, h : h + 1]
                )
                qrow = qt.tile([P, SC, DQ], BF16, tag="qrow", name=f"qrow{h}")
                for t in range(SC):
                    tsl = slice(t * P, (t + 1) * P)
                    scr = scrp.tile([P, P], BF16, tag="scr", name=f"sq{h}{t}")
                    nc.tensor.transpose(scr[:], q_sb[:, tsl], identb[:])
                    nc.scalar.activation(qrow[:, t, :], scr[:], AF.Identity)
                return qrow

            def q_stageB(h, qrow):
                qagg, qrst = ln_stats(qt, qrow, SC, DQ, 4, f"q{h}")
                xnq = qt.tile([P, SC, DQ], BF16, tag="xnq", name=f"xnq{h}")
                for t in range(SC):
                    nc.vector.tensor_scalar(
                        xnq[:, t, :], qrow[:, t, :],
                        qagg[:, t, 0:1], qrst[:, t, 0:1],
                        TT.subtract, TT.mult,
                    )
                z1 = qt.tile([P, SC, HALF], BF16, tag="qz1", name=f"qz1{h}")
                z2 = qt.tile([P, SC, HALF], BF16, tag="qz2", name=f"qz2{h}")
                qr = qt.tile([P, SC, DQ], BF16, tag="qr", name=f"qr{h}")
                nc.vector.tensor_tensor(
                    z1[:], xnq[:, :, :HALF], c1q[:], TT.mult
                )
                nc.vector.tensor_tensor(
                    z2[:], xnq[:, :, HALF:], s2nq[:], TT.mult
                )
                nc.vector.tensor_tensor(qr[:, :, :HALF], z1[:], z2[:], TT.add)
                nc.vector.tensor_tensor(
                    z1[:], xnq[:, :, :HALF], s1q[:], TT.mult
                )
                nc.vector.tensor_tensor(
                    z2[:], xnq[:, :, HALF:], c2q[:], TT.mult
                )
                nc.vector.tensor_tensor(qr[:, :, HALF:], z1[:], z2[:], TT.add)
                if has_rbq:
                    nc.vector.tensor_tensor(qr[:], qr[:], rbq[:], TT.add)
                for t in range(SC):
                    scb = qscr.tile([P, P], BF16, tag="qsc", name=f"sqb{h}{t}")
                    nc.tensor.transpose(scb[:], qr[:, t, :], identb[:])
                    nc.scalar.copy(qT[h][:, t * P : (t + 1) * P], scb[:])

            prev = None
            for h in range(H):
                qrow = q_stageA(h)
                if prev is not None:
                    q_stageB(*prev)
                prev = (h, qrow)
            q_stageB(*prev)

        # =========================================================
        # Attention: per head, PV of head h-1 interleaved into the
        # QK/softcap stream of head h to fill PE stalls
        # =========================================================
        wo_sb = wop.tile([P, DC, D], BF16)
        nc.sync.dma_start(wo_sb[:], wo)
        bor = wop.tile([P, D], F32)
        nc.sync.dma_start(bor[:], bor_t)
        yT_sb = wop.tile([P, DC, SQ], BF16)

        with (
            tc.tile_pool(name="att", bufs=2) as att,
            tc.tile_pool(name="apq", bufs=2, space="PSUM") as apq,
            tc.tile_pool(name="ay", bufs=2, space="PSUM") as ay,
        ):
            pts = {}

            def qk_group(h, jg):
                pq = apq.tile([P, 2, 512], F32, tag="pq", name=f"pq{h}{jg}")
                for c in range(2):
                    jc = jg * 2 + c
                    nc.tensor.matmul(
                        pq[:, c, :], identb[:], biasT_sb[:, jc, :],
                        start=True, stop=False,
                    )
                    nc.tensor.matmul(
                        pq[:, c, :],
                        kT_sb[:, jc * P : (jc + 1) * P], qT[h][:],
                        start=False, stop=True,
                    )
                nc.scalar.activation(
                    pq[:], pq[:], AF.Tanh, scale=1.0 / SOFTCAP
                )
                nc.scalar.activation(
                    pts[h][:, jg * 2 : jg * 2 + 2, :], pq[:],
                    AF.Exp, scale=SOFTCAP,
                )

            def pv_chunk(h, ic):
                y_ps = ay.tile([P, DV + 1], F32, tag="y_ps",
                               name=f"yps{h}{ic}")
                for jc in range(JC):
                    nc.tensor.matmul(
                        y_ps[:],
                        pts[h][:, jc, ic * P : (ic + 1) * P],
                        vrow_sb[:, jc, : DV + 1],
                        start=(jc == 0), stop=(jc == JC - 1),
                    )
                rcp = att.tile([P, 1], F32, tag="rcp", name=f"rcp{h}{ic}")
                nc.vector.reciprocal(rcp[:], y_ps[:, DV : DV + 1])
                nc.vector.tensor_scalar_mul(
                    yp[h // 2][:, ic, (h % 2) * DV : (h % 2 + 1) * DV],
                    y_ps[:, :DV], rcp[:, 0:1],
                )

            def pair_transposes(p):
                for sc in range(SC):
                    for fcl in range(3):
                        fc = 3 * p + fcl
                        scb = scrp.tile([P, P], BF16, tag="scr",
                                        name=f"so{sc}{fc}")
                        nc.tensor.transpose(
                            scb[:],
                            yp[p][:, sc, fcl * P : (fcl + 1) * P],
                            identb[:],
                        )
                        nc.vector.tensor_copy(
                            yT_sb[:, fc, sc * P : (sc + 1) * P], scb[:]
                        )

            for h in range(H):
                pts[h] = att.tile([P, JC, SQ], BF16, tag="pt", name=f"pt{h}")
                for jg in range(JC // 2):
                    qk_group(h, jg)
                    if h > 0 and jg % 2 == 1:
                        pv_chunk(h - 1, jg // 2)
                if h >= 2 and h % 2 == 0:
                    pair_transposes((h - 2) // 2)
            for ic in range(SC):
                pv_chunk(H - 1, ic)
            pair_transposes(3)

        # =========================================================
        # Output projection (bf16)
        # =========================================================
        with (
            tc.tile_pool(name="od", bufs=2) as od,
            tc.tile_pool(name="ops", bufs=2, space="PSUM") as ops,
        ):
            for sc in range(SC):
                o_ps = ops.tile([P, D], F32, tag="o_ps", name=f"ops{sc}")
                for fc in range(DC):
                    for n in range(D // 512):
                        nc.tensor.matmul(
                            o_ps[:, n * 512 : (n + 1) * 512],
                            yT_sb[:, fc, sc * P : (sc + 1) * P],
                            wo_sb[:, fc, n * 512 : (n + 1) * 512],
                            start=(fc == 0), stop=(fc == DC - 1),
                        )
                o_sb = od.tile([P, D], F32, tag="o_sb", name=f"osb{sc}")
                nc.vector.tensor_tensor(
                    o_sb[:], o_ps[:], bor[:], TT.add
                )
                nc.sync.dma_start(
                    out[sc * P : (sc + 1) * P, :], o_sb[:]
                )

    nc.compile()
    return nc


def _host_prep(inputs):
    import ml_dtypes

    f32 = np.float32
    bf16 = ml_dtypes.bfloat16
    x = np.asarray(inputs["x"], f32)
    bias = np.asarray(inputs["attention_bias"], f32)
    g1 = np.asarray(inputs["g1"], f32)
    b1 = np.asarray(inputs["b1"], f32)
    rr1 = np.asarray(inputs["rrms1"], f32)
    Wq = np.asarray(inputs["Wq"], f32)
    Wk = np.asarray(inputs["Wk"], f32)
    Wv = np.asarray(inputs["Wv"], f32)
    qg = np.asarray(inputs["qg"], f32)
    qb = np.asarray(inputs["qb"], f32)
    kg = np.asarray(inputs["kg"], f32)
    kb = np.asarray(inputs["kb"], f32)
    vg = np.asarray(inputs["vg"], f32)
    vb = np.asarray(inputs["vb"], f32)
    Wo = np.asarray(inputs["Wo"], f32)
    bo = np.asarray(inputs["bo"], f32)
    g2 = np.asarray(inputs["g2"], f32)
    b2 = np.asarray(inputs["b2"], f32)
    rr2 = np.asarray(inputs["rrms2"], f32)

    scale1 = (g1 * (1.0 / np.sqrt(rr1 + EPS_RMS))).astype(f32)
    Wq_e = (Wq * scale1[:, None]).astype(f32)
    Wk_e = (Wk * scale1[:, None]).astype(f32)
    Wv_e = (Wv * scale1[:, None]).astype(f32)
    bq_row = (b1 @ Wq).astype(f32)      # [H*DQ]
    bk_row = (b1 @ Wk).astype(f32)      # [DK]
    bv_row = (b1 @ Wv).astype(f32)      # [DV]
    sc_q = f32(DQ) ** f32(-0.5)
    qg_e = (qg * sc_q).astype(f32)
    qb_e = (qb * sc_q).astype(f32)

    # v-affine folded through attention into Wo/bo; rms2 folded too
    scale2 = (g2 * (1.0 / np.sqrt(rr2 + EPS_RMS))).astype(f32)
    vg_rep = np.tile(vg, H)                      # [H*DV]
    Wo_e = (Wo * vg_rep[:, None] * scale2[None, :]).astype(f32)
    vb_fold = (np.tile(vb, H) @ Wo).astype(f32)  # [D]
    bo_e = ((bo + vb_fold) * scale2 + b2).astype(f32)

    freqs = (
        1.0 / (ROPE_BASE ** (np.arange(HALF, dtype=f32) / HALF))
    ).astype(f32)
    ang = np.arange(S, dtype=f32)[:, None] * freqs[None, :]
    cos = np.cos(ang).astype(f32)                        # [S, 64]
    sin = np.sin(ang).astype(f32)

    # rope tables with gamma folded (and DQ^-0.5 for q)
    c1k = (cos * kg[None, :HALF]).astype(bf16)
    s2nk = (-sin * kg[None, HALF:]).astype(bf16)
    s1k = (sin * kg[None, :HALF]).astype(bf16)
    c2k = (cos * kg[None, HALF:]).astype(bf16)

    # rope'd beta tables (rope(b) by position)
    rbk_f = np.concatenate(
        [cos * kb[None, :HALF] - sin * kb[None, HALF:],
         sin * kb[None, :HALF] + cos * kb[None, HALF:]], axis=1
    ).astype(f32)
    rbq_f = np.concatenate(
        [cos * qb_e[None, :HALF] - sin * qb_e[None, HALF:],
         sin * qb_e[None, :HALF] + cos * qb_e[None, HALF:]], axis=1
    ).astype(f32)
    has_rbk = bool(np.any(rbk_f))
    has_rbq = bool(np.any(rbq_f))

    def dev3(a, n):
        """[n*P, W] row-major -> [P, n, W] device layout, contiguous."""
        return np.ascontiguousarray(
            a.reshape(n, P, a.shape[-1]).transpose(1, 0, 2)
        )

    rep = lambda v: np.ascontiguousarray(
        np.broadcast_to(v[None, :], (P, v.shape[0]))
    )
    packf = np.zeros((P, 140), f32)
    packf[:, 0:P] = np.eye(P, dtype=f32)
    packf[:, 128] = bk_row
    packf[:, 129] = bv_row[:P]
    packf[: DV - P, 130] = bv_row[P:]
    packf[:, 131:139] = bq_row.reshape(H, DQ).T[:, :]  # wait shape
    # bq per (d, h): [P, H]
    packf[:, 131:139] = bq_row.reshape(H, DQ).T
    shared = {
        "c1k": dev3(c1k, JC),
        "s2nk": dev3(s2nk, JC),
        "s1k": dev3(s1k, JC),
        "c2k": dev3(c2k, JC),
        "wq": dev3(Wq_e, DC),
        "wk": dev3(Wk_e, DC),
        "wva": dev3(Wv_e[:, :P], DC),
        "wvb": dev3(Wv_e[:, P:], DC),
        "wo": dev3(Wo_e.astype(bf16), DC),
        "packf": packf,
        "bor": rep(bo_e),
        "identb": np.eye(P, dtype=bf16),
    }
    if has_rbk:
        shared["rbk"] = dev3(rbk_f.astype(bf16), JC)

    xTs = [np.ascontiguousarray(x[b].T) for b in range(B)]
    in_maps = []
    for c in range(NCORES):
        b = c // 4
        s0 = (c % 4) * SQ
        m = dict(shared)
        m["xT"] = xTs[b]
        m["xTq"] = dev3(xTs[b][:, s0 : s0 + SQ], DC)
        m["biasT"] = dev3(bias[0, 0, s0 : s0 + SQ, :].T.astype(bf16), JC)
        m["c1q"] = dev3(
            (cos[s0 : s0 + SQ] * qg_e[None, :HALF]).astype(bf16), SC
        )
        m["s2nq"] = dev3(
            (-sin[s0 : s0 + SQ] * qg_e[None, HALF:]).astype(bf16), SC
        )
        m["s1q"] = dev3(
            (sin[s0 : s0 + SQ] * qg_e[None, :HALF]).astype(bf16), SC
        )
        m["c2q"] = dev3(
            (cos[s0 : s0 + SQ] * qg_e[None, HALF:]).astype(bf16), SC
        )
        if has_rbq:
            m["rbq"] = dev3(rbq_f[s0 : s0 + SQ].astype(bf16), SC)
        in_maps.append(m)
    return in_maps, has_rbq, has_rbk


_NC_CACHE = {}


def _get_nc(has_rbq=False, has_rbk=False):
    key = (has_rbq, has_rbk)
    if key not in _NC_CACHE:
        _NC_CACHE[key] = build_program(has_rbq, has_rbk)
    return _NC_CACHE[key]


def kernel(**inputs) -> np.ndarray:
    in_maps, has_rbq, has_rbk = _host_prep(inputs)
    nc = _get_nc(has_rbq, has_rbk)
    res = bass_utils.run_bass_kernel_spmd(
        nc, in_maps, core_ids=list(range(NCORES))
    )
    outs = res.results
    full = np.empty((B, S, D), np.float32)
    for c in range(NCORES):
        b = c // 4
        s0 = (c % 4) * SQ
        full[b, s0 : s0 + SQ, :] = outs[c]["out"]
    return full


if __name__ == "__main__":
    nc = _get_nc()
    print("build + compile OK")


# revision 18
# speedup vs baseline: 1.0917x; 1.0917x over previous
"""Trainium2 Bass kernel for an MQA attention block (8 q-heads, shared K/V).

Sharding: 8 cores; core c -> batch b=c//4, query rows s0=(c%4)*512 .. +512,
all 8 heads.  K/V (full sequence, per batch) computed redundantly per core.

v2 design notes:
- All LN affines folded on host: rms1 into Wq/Wk/Wv, q/k gammas (and DQ^-0.5)
  into the rope cos/sin tables, v gamma/beta through attention into Wo/bo,
  rms2 into Wo/bo.
- LN stats via ScalarE: the PSUM->SBUF drain copy doubles as the row-sum
  (activation Identity + accum_out), a Square activation gives sum-of-squares.
- Attention bias is preloaded into PSUM by the PE itself (matmul with an
  identity stationary and the bias as moving operand, start=True), so the
  QK matmuls accumulate on top -- no DVE bias add.
- Rope'd q/k, softmax probs, v rows, y and Wo all run in bf16 (PE full rate
  at any moving width); projections and logits accumulate in fp32.
- Softmax denominator from a ones-column appended to v (no max subtraction
  needed: logits softcapped to +-5).
"""

import os
import sys

for _p in ("/opt/trn_rl_repo",):
    if _p not in sys.path and os.path.isdir(_p):
        sys.path.insert(0, _p)

import numpy as np
from contextlib import ExitStack

import concourse.bass as bass
import concourse.mybir as mybir
import concourse.tile as tile
from concourse import bacc
from concourse import bass_utils

F32 = mybir.dt.float32
F32R = mybir.dt.float32r
BF16 = mybir.dt.bfloat16

B, S, D = 2, 2048, 1536
H, DQ, DK, DV = 8, 128, 128, 192
P = 128
SQ = S // 4          # 512 query rows per core
DC = D // P          # 12 contraction chunks
JC = S // P          # 16 key chunks
SC = SQ // P         # 4 query-row chunks
NCORES = 8
EPS_RMS = 1e-6
EPS_LN = 1e-5
SOFTCAP = 5.0
ROPE_BASE = 8192.0
HALF = DQ // 2
VW = 256             # vrow inner stride (bf16); cols 0:192 v, 192 ones
JH = S // 2          # columns per half in kv projection


def _r(ap):
    return ap.bitcast(F32R)


def build_program(has_rbq=False, has_rbk=False):
    nc = bacc.Bacc(
        "TRN2", target_bir_lowering=False, debug=False, num_devices=NCORES
    )

    def din(name, shape, dt=F32):
        return nc.dram_tensor(name, list(shape), dt, kind="ExternalInput").ap()

    # per-core inputs (host pre-arranged to device layouts, contiguous)
    xT = din("xT", (D, S))
    xTq = din("xTq", (P, DC, SQ))
    biasT = din("biasT", (P, JC, SQ), BF16)
    c1q_t = din("c1q", (P, SC, HALF), BF16)
    s2nq_t = din("s2nq", (P, SC, HALF), BF16)
    s1q_t = din("s1q", (P, SC, HALF), BF16)
    c2q_t = din("c2q", (P, SC, HALF), BF16)
    # shared (replicated) inputs
    c1k_t = din("c1k", (P, JC, HALF), BF16)
    s2nk_t = din("s2nk", (P, JC, HALF), BF16)
    s1k_t = din("s1k", (P, JC, HALF), BF16)
    c2k_t = din("c2k", (P, JC, HALF), BF16)
    wq = din("wq", (P, DC, H * DQ))
    wk = din("wk", (P, DC, DK))
    wva_t = din("wva", (P, DC, P))
    wvb_t = din("wvb", (P, DC, DV - P))
    wo = din("wo", (P, DC, D), BF16)
    packf_t = din("packf", (P, 140))  # identf|bk|bvA|bvB|bqh packed
    bor_t = din("bor", (P, D))    # row-replicated output bias (all folds)
    identb_t = din("identb", (P, P), BF16)
    if has_rbq:
        rbq_t = din("rbq", (P, SC, DQ), BF16)
    if has_rbk:
        rbk_t = din("rbk", (P, JC, DK), BF16)
    out = nc.dram_tensor("out", [SQ, D], F32, kind="ExternalOutput").ap()

    TT = mybir.AluOpType
    AF = mybir.ActivationFunctionType
    AX = mybir.AxisListType

    with tile.TileContext(nc) as tc, ExitStack() as ctx:
        const = ctx.enter_context(tc.tile_pool(name="const", bufs=1))
        persist = ctx.enter_context(tc.tile_pool(name="persist", bufs=1))
        scrp_cm = tc.tile_pool(name="scrp", bufs=2, space="PSUM")
        scrp = scrp_cm.__enter__()

        # ---- constants (DMAs emitted after the kv weight DMAs)
        packf = const.tile([P, 140], F32)
        bk_sb = packf[:, 128:129]
        bvA = packf[:, 129:130]
        bvB = packf[0 : DV - P, 130:131]
        bqh_sb = packf[:, 131:139]
        identb = const.tile([P, P], BF16)
        eps_sb = const.tile([P, 1], F32)
        nc.vector.memset(eps_sb[:], EPS_LN)

        def load_tab(t, n, nm):
            tt = const.tile([P, n, HALF], BF16, tag=nm, name=nm)
            nc.sync.dma_start(tt[:], t)
            return tt

        if has_rbk:
            rbk = const.tile([P, JC, DK], BF16)
            nc.sync.dma_start(rbk[:], rbk_t)
        if has_rbq:
            rbq = const.tile([P, SC, DQ], BF16)
            nc.sync.dma_start(rbq[:], rbq_t)


        # persistent activations
        kT_sb = persist.tile([P, S], BF16)           # rope'd k, [dk, s]
        vrow_sb = persist.tile([P, JC, VW], BF16)    # v rows + ones col
        nc.vector.memset(vrow_sb[:, :, DV : DV + 1], 1.0)
        qT = [
            persist.tile([P, SQ], BF16, tag=f"q{h}", name=f"qT{h}")
            for h in range(H)
        ]
        yp = [
            persist.tile([P, SC, 2 * DV], BF16, tag=f"yp{p}", name=f"yp{p}")
            for p in range(4)
        ]

        # =========================================================
        # KV phase
        # =========================================================
        def ln_stats(pool, rows, G, W, grp, nm):
            """bn_stats/aggr over [P, G, W] rows -> (agg [P,G,2], rst [P,G,1])
            grp chunks per bn_stats call (grp*W <= 512)."""
            st6 = pool.tile([P, G, 6], F32, tag=nm + "st6", name=nm + "st6")
            agg = pool.tile([P, G, 2], F32, tag=nm + "agg", name=nm + "agg")
            for t in range(G):
                nc.vector.bn_stats(st6[:, t, :], rows[:, t, :])
                nc.vector.bn_aggr(agg[:, t, :], st6[:, t, :])
            rst = pool.tile([P, G, 1], F32, tag=nm + "rst", name=nm + "rst")
            nc.scalar.activation(
                rst[:], agg[:, :, 1:2], AF.Sqrt, bias=eps_sb[:, 0:1]
            )
            nc.vector.reciprocal(rst[:], rst[:])
            return agg, rst

        with (
            tc.tile_pool(name="kvw", bufs=1) as kvw,
            tc.tile_pool(name="kvx", bufs=3) as kvx,
            tc.tile_pool(name="kvd", bufs=2) as kvd,
            tc.tile_pool(name="kvt", bufs=2) as kvt,
            tc.tile_pool(name="kvps", bufs=1, space="PSUM") as kvps,
        ):
            wk_sb = kvw.tile([P, DC, DK], F32)
            nc.sync.dma_start(_r(wk_sb[:]), _r(wk))
            wva = kvw.tile([P, DC, P], F32)
            wvb = kvw.tile([P, DC, DV - P], F32)
            nc.sync.dma_start(_r(wva[:]), _r(wva_t))
            nc.sync.dma_start(_r(wvb[:]), _r(wvb_t))

            krs = []
            for jh in range(2):
                j0 = jh * JH
                kps = kvps.tile([P, JH], F32, tag="kps")
                vaps = kvps.tile([P, JH], F32, tag="vaps")
                vbps = kvps.tile([DV - P, JH], F32, tag="vbps")
                for dc in range(DC):
                    xt = kvx.tile([P, JH], F32, tag="xt")
                    nc.sync.dma_start(
                        _r(xt[:]), _r(xT[dc * P : (dc + 1) * P, j0 : j0 + JH])
                    )
                    for n in range(JH // 512):
                        cs = slice(n * 512, (n + 1) * 512)
                        nc.tensor.matmul(
                            kps[:, cs], _r(wk_sb[:, dc, :]), _r(xt[:, cs]),
                            start=(dc == 0), stop=(dc == DC - 1),
                        )
                        nc.tensor.matmul(
                            vaps[:, cs], _r(wva[:, dc, :]), _r(xt[:, cs]),
                            start=(dc == 0), stop=(dc == DC - 1),
                        )
                        nc.tensor.matmul(
                            vbps[:, cs], _r(wvb[:, dc, :]), _r(xt[:, cs]),
                            start=(dc == 0), stop=(dc == DC - 1),
                        )

                if jh == 0:
                    nc.sync.dma_start(packf[:], packf_t)
                    nc.sync.dma_start(identb[:], identb_t)
                    c1k = load_tab(c1k_t, JC, "c1k")
                    s2nk = load_tab(s2nk_t, JC, "s2nk")
                    s1k = load_tab(s1k_t, JC, "s1k")
                    c2k = load_tab(c2k_t, JC, "c2k")

                # drains with folded rms1 bias
                k_sb = kvd.tile([P, JH], BF16, tag="k_sb")
                nc.scalar.activation(
                    k_sb[:], kps[:], AF.Identity, bias=bk_sb
                )
                va_sb = kvd.tile([P, JH], BF16, tag="va_sb")
                nc.scalar.activation(
                    va_sb[:], vaps[:], AF.Identity, bias=bvA
                )
                vb_sb = kvd.tile([DV - P, JH], BF16, tag="vb_sb")
                nc.scalar.activation(
                    vb_sb[:], vbps[:], AF.Identity, bias=bvB
                )

                # ---- k chunks: transpose -> rows, LN stats, rope
                krow = kvt.tile([P, 8, DK], BF16, tag="krow")
                for t in range(8):
                    tsl = slice(t * P, (t + 1) * P)
                    scr = scrp.tile([P, P], BF16, tag="scr", name=f"sk{jh}{t}")
                    nc.tensor.transpose(scr[:], k_sb[:, tsl], identb[:])
                    nc.scalar.activation(krow[:, t, :], scr[:], AF.Identity)
                kagg, krst = ln_stats(kvt, krow, 8, DK, 4, "k")
                xn = kvt.tile([P, 8, DK], BF16, tag="xn")
                for t in range(8):
                    nc.vector.tensor_scalar(
                        xn[:, t, :], krow[:, t, :],
                        kagg[:, t, 0:1], krst[:, t, 0:1],
                        TT.subtract, TT.mult,
                    )
                # batched rope over the 8 chunks
                jcs = slice(jh * 8, jh * 8 + 8)
                z1 = kvt.tile([P, 8, HALF], BF16, tag="z1")
                z2 = kvt.tile([P, 8, HALF], BF16, tag="z2")
                kr = kvt.tile([P, 8, DK], BF16, tag="kr")
                nc.vector.tensor_tensor(
                    z1[:], xn[:, :, :HALF], c1k[:, jcs, :], TT.mult
                )
                nc.vector.tensor_tensor(
                    z2[:], xn[:, :, HALF:], s2nk[:, jcs, :], TT.mult
                )
                nc.vector.tensor_tensor(kr[:, :, :HALF], z1[:], z2[:], TT.add)
                nc.vector.tensor_tensor(
                    z1[:], xn[:, :, :HALF], s1k[:, jcs, :], TT.mult
                )
                nc.vector.tensor_tensor(
                    z2[:], xn[:, :, HALF:], c2k[:, jcs, :], TT.mult
                )
                nc.vector.tensor_tensor(kr[:, :, HALF:], z1[:], z2[:], TT.add)
                if has_rbk:
                    nc.vector.tensor_tensor(
                        kr[:], kr[:], rbk[:, jcs, :], TT.add
                    )
                krs.append(kr)

                # ---- v chunks
                vrowf = kvt.tile([P, 8, DV], BF16, tag="vrowf")
                for t in range(8):
                    tsl = slice(t * P, (t + 1) * P)
                    scv = scrp.tile([P, P + 64], BF16, tag="scr",
                                    name=f"sv{jh}{t}")
                    nc.tensor.transpose(scv[:, :P], va_sb[:, tsl], identb[:])
                    nc.tensor.transpose(
                        scv[:, P:], vb_sb[:, tsl],
                        identb[: DV - P, : DV - P],
                    )
                    nc.scalar.activation(
                        vrowf[:, t, :], scv[:, :DV], AF.Identity
                    )
                vagg, vrst = ln_stats(kvt, vrowf, 8, DV, 2, "v")
                for t in range(8):
                    jc = jh * 8 + t
                    nc.vector.tensor_scalar(
                        vrow_sb[:, jc, :DV], vrowf[:, t, :],
                        vagg[:, t, 0:1], vrst[:, t, 0:1],
                        TT.subtract, TT.mult,
                    )

            # deferred k back-transposes (keeps the PE stream dense)
            for jh in range(2):
                for t in range(8):
                    jc = jh * 8 + t
                    scb = scrp.tile([P, P], BF16, tag="scr", name=f"sb{jh}{t}")
                    nc.tensor.transpose(scb[:], krs[jh][:, t, :], identb[:])
                    nc.vector.tensor_copy(
                        kT_sb[:, jc * P : (jc + 1) * P], scb[:]
                    )

        # attention constants loaded during Q phase
        wop = ctx.enter_context(tc.tile_pool(name="wop", bufs=1))
        biasT_sb = wop.tile([P, JC, SQ], BF16)
        nc.sync.dma_start(biasT_sb[:], biasT)

        # =========================================================
        # Q phase: software-pipelined stages so PE stays dense
        # =========================================================
        with (
            tc.tile_pool(name="qx", bufs=1) as qx,
            tc.tile_pool(name="qw", bufs=2) as qw,
            tc.tile_pool(name="qt", bufs=2) as qt,
            tc.tile_pool(name="qps", bufs=2, space="PSUM") as qps,
            tc.tile_pool(name="qscr", bufs=2, space="PSUM") as qscr,
        ):
            xtq_sb = qx.tile([P, DC, SQ], F32)
            nc.sync.dma_start(_r(xtq_sb[:]), _r(xTq))
            c1q = load_tab(c1q_t, SC, "c1q")
            s2nq = load_tab(s2nq_t, SC, "s2nq")
            s1q = load_tab(s1q_t, SC, "s1q")
            c2q = load_tab(c2q_t, SC, "c2q")

            def q_stageA(h):
                wqh = qw.tile([P, DC, DQ], F32, tag="wqh", name=f"wqh{h}")
                nc.sync.dma_start(
                    _r(wqh[:]), _r(wq[:, :, h * DQ : (h + 1) * DQ])
                )
                q_ps = qps.tile([P, SQ], F32, tag="q_ps", name=f"qps{h}")
                for dc in range(DC):
                    nc.tensor.matmul(
                        q_ps[:], _r(wqh[:, dc, :]), _r(xtq_sb[:, dc, :]),
                        start=(dc == 0), stop=(dc == DC - 1),
                    )
                q_sb = qt.tile([P, SQ], BF16, tag="q_sb", name=f"qsb{h}")
                nc.scalar.activation(
                    q_sb[:], q_ps[:], AF.Identity, bias=bqh_sb[:, h : h + 1]
                )
                qrow = qt.tile([P, SC, DQ], BF16, tag="qrow", name=f"qrow{h}")
                for t in range(SC):
                    tsl = slice(t * P, (t + 1) * P)
                    scr = scrp.tile([P, P], BF16, tag="scr", name=f"sq{h}{t}")
                    nc.tensor.transpose(scr[:], q_sb[:, tsl], identb[:])
                    nc.scalar.activation(qrow[:, t, :], scr[:], AF.Identity)
                return qrow

            def q_stageB(h, qrow):
                qagg, qrst = ln_stats(qt, qrow, SC, DQ, 4, f"q{h}")
                xnq = qt.tile([P, SC, DQ], BF16, tag="xnq", name=f"xnq{h}")
                for t in range(SC):
                    nc.vector.tensor_scalar(
                        xnq[:, t, :], qrow[:, t, :],
                        qagg[:, t, 0:1], qrst[:, t, 0:1],
                        TT.subtract, TT.mult,
                    )
                z1 = qt.tile([P, SC, HALF], BF16, tag="qz1", name=f"qz1{h}")
                z2 = qt.tile([P, SC, HALF], BF16, tag="qz2", name=f"qz2{h}")
                qr = qt.tile([P, SC, DQ], BF16, tag="qr", name=f"qr{h}")
                nc.vector.tensor_tensor(
                    z1[:], xnq[:, :, :HALF], c1q[:], TT.mult
                )
                nc.vector.tensor_tensor(
                    z2[:], xnq[:, :, HALF:], s2nq[:], TT.mult
                )
                nc.vector.tensor_tensor(qr[:, :, :HALF], z1[:], z2[:], TT.add)
                nc.vector.tensor_tensor(
                    z1[:], xnq[:, :, :HALF], s1q[:], TT.mult
                )
                nc.vector.tensor_tensor(
                    z2[:], xnq[:, :, HALF:], c2q[:], TT.mult
                )
                nc.vector.tensor_tensor(qr[:, :, HALF:], z1[:], z2[:], TT.add)
                if has_rbq:
                    nc.vector.tensor_tensor(qr[:], qr[:], rbq[:], TT.add)
                for t in range(SC):
                    scb = qscr.tile([P, P], BF16, tag="qsc", name=f"sqb{h}{t}")
                    nc.tensor.transpose(scb[:], qr[:, t, :], identb[:])
                    nc.scalar.copy(qT[h][:, t * P : (t + 1) * P], scb[:])

            prev = None
            for h in range(H):
                qrow = q_stageA(h)
                if prev is not None:
                    q_stageB(*prev)
                prev = (h, qrow)
            q_stageB(*prev)

        scrp_cm.__exit__(None, None, None)

        # =========================================================
        # Attention: per head, PV of head h-1 interleaved into the
        # QK/softcap stream of head h to fill PE stalls
        # =========================================================
        wo_sb = wop.tile([P, DC, D], BF16)
        nc.sync.dma_start(wo_sb[:], wo)
        bor = wop.tile([P, D], F32)
        nc.sync.dma_start(bor[:], bor_t)
        yT_sb = wop.tile([P, DC, SQ], BF16)

        with (
            tc.tile_pool(name="att", bufs=2) as att,
            tc.tile_pool(name="apq", bufs=3, space="PSUM") as apq,
            tc.tile_pool(name="ay", bufs=2, space="PSUM") as ay,
        ):
            pts = {}

            def qk_group(h, jg):
                pq = apq.tile([P, 2, 512], F32, tag="pq", name=f"pq{h}{jg}")
                for c in range(2):
                    jc = jg * 2 + c
                    nc.tensor.matmul(
                        pq[:, c, :], identb[:], biasT_sb[:, jc, :],
                        start=True, stop=False,
                    )
                    nc.tensor.matmul(
                        pq[:, c, :],
                        kT_sb[:, jc * P : (jc + 1) * P], qT[h][:],
                        start=False, stop=True,
                    )
                nc.scalar.activation(
                    pq[:], pq[:], AF.Tanh, scale=1.0 / SOFTCAP
                )
                nc.scalar.activation(
                    pts[h][:, jg * 2 : jg * 2 + 2, :], pq[:],
                    AF.Exp, scale=SOFTCAP,
                )

            def pv_chunk(h, ic):
                y_ps = ay.tile([P, DV + 1], F32, tag="y_ps",
                               name=f"yps{h}{ic}")
                for jc in range(JC):
                    nc.tensor.matmul(
                        y_ps[:],
                        pts[h][:, jc, ic * P : (ic + 1) * P],
                        vrow_sb[:, jc, : DV + 1],
                        start=(jc == 0), stop=(jc == JC - 1),
                    )
                rcp = att.tile([P, 1], F32, tag="rcp", name=f"rcp{h}{ic}")
                nc.vector.reciprocal(rcp[:], y_ps[:, DV : DV + 1])
                nc.vector.tensor_scalar_mul(
                    yp[h // 2][:, ic, (h % 2) * DV : (h % 2 + 1) * DV],
                    y_ps[:, :DV], rcp[:, 0:1],
                )

            for h in range(H):
                pts[h] = att.tile([P, JC, SQ], BF16, tag="pt", name=f"pt{h}")
                for jg in range(JC // 2):
                    qk_group(h, jg)
                    if h > 0 and jg % 2 == 1:
                        pv_chunk(h - 1, jg // 2)
            for ic in range(SC):
                pv_chunk(H - 1, ic)

        # =========================================================
        # Output projection (bf16)
        # =========================================================
        with (
            tc.tile_pool(name="od", bufs=2) as od,
            tc.tile_pool(name="ops", bufs=2, space="PSUM") as ops,
        ):
            for p in range(4):
                for sc in range(SC):
                    for fcl in range(3):
                        fc = 3 * p + fcl
                        scb = ops.tile([P, P], BF16, tag="scb",
                                       name=f"so{sc}{fc}")
                        nc.tensor.transpose(
                            scb[:],
                            yp[p][:, sc, fcl * P : (fcl + 1) * P],
                            identb[:],
                        )
                        nc.vector.tensor_copy(
                            yT_sb[:, fc, sc * P : (sc + 1) * P], scb[:]
                        )
            for sc in range(SC):
                o_ps = ops.tile([P, D], F32, tag="o_ps", name=f"ops{sc}")
                for fc in range(DC):
                    for n in range(D // 512):
                        nc.tensor.matmul(
                            o_ps[:, n * 512 : (n + 1) * 512],
                            yT_sb[:, fc, sc * P : (sc + 1) * P],
                            wo_sb[:, fc, n * 512 : (n + 1) * 512],
                            start=(fc == 0), stop=(fc == DC - 1),
                        )
                o_sb = od.tile([P, D], F32, tag="o_sb", name=f"osb{sc}")
                nc.vector.tensor_tensor(
                    o_sb[:], o_ps[:], bor[:], TT.add
                )
                nc.sync.dma_start(
                    out[sc * P : (sc + 1) * P, :], o_sb[:]
                )

    nc.compile()
    return nc


def _host_prep(inputs):
    import ml_dtypes

    f32 = np.float32
    bf16 = ml_dtypes.bfloat16
    x = np.asarray(inputs["x"], f32)
    bias = np.asarray(inputs["attention_bias"], f32)
    g1 = np.asarray(inputs["g1"], f32)
    b1 = np.asarray(inputs["b1"], f32)
    rr1 = np.asarray(inputs["rrms1"], f32)
    Wq = np.asarray(inputs["Wq"], f32)
    Wk = np.asarray(inputs["Wk"], f32)
    Wv = np.asarray(inputs["Wv"], f32)
    qg = np.asarray(inputs["qg"], f32)
    qb = np.asarray(inputs["qb"], f32)
    kg = np.asarray(inputs["kg"], f32)
    kb = np.asarray(inputs["kb"], f32)
    vg = np.asarray(inputs["vg"], f32)
    vb = np.asarray(inputs["vb"], f32)
    Wo = np.asarray(inputs["Wo"], f32)
    bo = np.asarray(inputs["bo"], f32)
    g2 = np.asarray(inputs["g2"], f32)
    b2 = np.asarray(inputs["b2"], f32)
    rr2 = np.asarray(inputs["rrms2"], f32)

    scale1 = (g1 * (1.0 / np.sqrt(rr1 + EPS_RMS))).astype(f32)
    Wq_e = (Wq * scale1[:, None]).astype(f32)
    Wk_e = (Wk * scale1[:, None]).astype(f32)
    Wv_e = (Wv * scale1[:, None]).astype(f32)
    bq_row = (b1 @ Wq).astype(f32)      # [H*DQ]
    bk_row = (b1 @ Wk).astype(f32)      # [DK]
    bv_row = (b1 @ Wv).astype(f32)      # [DV]
    sc_q = f32(DQ) ** f32(-0.5)
    qg_e = (qg * sc_q).astype(f32)
    qb_e = (qb * sc_q).astype(f32)

    # v-affine folded through attention into Wo/bo; rms2 folded too
    scale2 = (g2 * (1.0 / np.sqrt(rr2 + EPS_RMS))).astype(f32)
    vg_rep = np.tile(vg, H)                      # [H*DV]
    Wo_e = (Wo * vg_rep[:, None] * scale2[None, :]).astype(f32)
    vb_fold = (np.tile(vb, H) @ Wo).astype(f32)  # [D]
    bo_e = ((bo + vb_fold) * scale2 + b2).astype(f32)

    freqs = (
        1.0 / (ROPE_BASE ** (np.arange(HALF, dtype=f32) / HALF))
    ).astype(f32)
    ang = np.arange(S, dtype=f32)[:, None] * freqs[None, :]
    cos = np.cos(ang).astype(f32)                        # [S, 64]
    sin = np.sin(ang).astype(f32)

    # rope tables with gamma folded (and DQ^-0.5 for q)
    c1k = (cos * kg[None, :HALF]).astype(bf16)
    s2nk = (-sin * kg[None, HALF:]).astype(bf16)
    s1k = (sin * kg[None, :HALF]).astype(bf16)
    c2k = (cos * kg[None, HALF:]).astype(bf16)

    # rope'd beta tables (rope(b) by position)
    rbk_f = np.concatenate(
        [cos * kb[None, :HALF] - sin * kb[None, HALF:],
         sin * kb[None, :HALF] + cos * kb[None, HALF:]], axis=1
    ).astype(f32)
    rbq_f = np.concatenate(
        [cos * qb_e[None, :HALF] - sin * qb_e[None, HALF:],
         sin * qb_e[None, :HALF] + cos * qb_e[None, HALF:]], axis=1
    ).astype(f32)
    has_rbk = bool(np.any(rbk_f))
    has_rbq = bool(np.any(rbq_f))

    def dev3(a, n):
        """[n*P, W] row-major -> [P, n, W] device layout, contiguous."""
        return np.ascontiguousarray(
            a.reshape(n, P, a.shape[-1]).transpose(1, 0, 2)
        )

    rep = lambda v: np.ascontiguousarray(
        np.broadcast_to(v[None, :], (P, v.shape[0]))
    )
    packf = np.zeros((P, 140), f32)
    packf[:, 0:P] = np.eye(P, dtype=f32)
    packf[:, 128] = bk_row
    packf[:, 129] = bv_row[:P]
    packf[: DV - P, 130] = bv_row[P:]
    packf[:, 131:139] = bq_row.reshape(H, DQ).T[:, :]  # wait shape
    # bq per (d, h): [P, H]
    packf[:, 131:139] = bq_row.reshape(H, DQ).T
    shared = {
        "c1k": dev3(c1k, JC),
        "s2nk": dev3(s2nk, JC),
        "s1k": dev3(s1k, JC),
        "c2k": dev3(c2k, JC),
        "wq": dev3(Wq_e, DC),
        "wk": dev3(Wk_e, DC),
        "wva": dev3(Wv_e[:, :P], DC),
        "wvb": dev3(Wv_e[:, P:], DC),
        "wo": dev3(Wo_e.astype(bf16), DC),
        "packf": packf,
        "bor": rep(bo_e),
        "identb": np.eye(P, dtype=bf16),
    }
    if has_rbk:
        shared["rbk"] = dev3(rbk_f.astype(bf16), JC)

    xTs = [np.ascontiguousarray(x[b].T) for b in range(B)]
    in_maps = []
    for c in range(NCORES):
        b = c // 4
        s0 = (c % 4) * SQ
        m = dict(shared)
        m["xT"] = xTs[b]
        m["xTq"] = dev3(xTs[b][:, s0 : s0 + SQ], DC)
        m["biasT"] = dev3(bias[0, 0, s0 : s0 + SQ, :].T.astype(bf16), JC)
        m["c1q"] = dev3(
            (cos[s0 : s0 + SQ] * qg_e[None, :HALF]).astype(bf16), SC
        )
        m["s2nq"] = dev3(
            (-sin[s0 : s0 + SQ] * qg_e[None, HALF:]).astype(bf16), SC
        )
        m["s1q"] = dev3(
            (sin[s0 : s0 + SQ] * qg_e[None, :HALF]).astype(bf16), SC
        )
        m["c2q"] = dev3(
            (cos[s0 : s0 + SQ] * qg_e[None, HALF:]).astype(bf16), SC
        )
        if has_rbq:
            m["rbq"] = dev3(rbq_f[s0 : s0 + SQ].astype(bf16), SC)
        in_maps.append(m)
    return in_maps, has_rbq, has_rbk


_NC_CACHE = {}


def _get_nc(has_rbq=False, has_rbk=False):
    key = (has_rbq, has_rbk)
    if key not in _NC_CACHE:
        _NC_CACHE[key] = build_program(has_rbq, has_rbk)
    return _NC_CACHE[key]


def kernel(**inputs) -> np.ndarray:
    in_maps, has_rbq, has_rbk = _host_prep(inputs)
    nc = _get_nc(has_rbq, has_rbk)
    res = bass_utils.run_bass_kernel_spmd(
        nc, in_maps, core_ids=list(range(NCORES))
    )
    outs = res.results
    full = np.empty((B, S, D), np.float32)
    for c in range(NCORES):
        b = c // 4
        s0 = (c % 4) * SQ
        full[b, s0 : s0 + SQ, :] = outs[c]["out"]
    return full


if __name__ == "__main__":
    nc = _get_nc()
    print("build + compile OK")
